# revision 27
# baseline (speedup 1.0000x reference)
"""Trainium2 Bass kernel for nn_BiGNN (gnn_message_passing).

Math: p_i = max_k relu(bn_i(feat_i[idx_i] @ Wg_i)); out = relu(bn_o(cat @ Wout)).
BN is folded on the host (sign into Wg columns, |scale| into head weights):
  z_i = feat_i @ (Wg_i * sign(s_i)); m_i = max_k z_i[idx_i]
  out = relu(featL @ WA + relu(m1+b1') @ WB + relu(m2+b2') @ WC + bo)

Strategy (8 cores, data-parallel over the 50k target voxels, 6656 padded
targets per core): the neighbor gather is done ON THE HOST — the device
receives per-core pre-gathered, bf16, channel-major "expanded" feature
tables E[(s2 ch 0..63 | s1 ch 0..31), col] where col encodes (t, k) in the
exact order the device consumes.  No dma_gather, no index tables, no
transposes on device:

  per 2048-col step and scale: 4 matmuls with the (BN-folded) Wg STATIONARY
  stream E columns into one 2-bank PSUM group [128, 2x512] (two 64-channel
  target-halves stacked on partitions so all 128 DVE lanes work).  Pooling
  over k=16 column groups is split ~30/70 between a direct DVE tensor_reduce
  from PSUM (1x microcode) and an ACT PSUM->bf16 copy followed by a 2x-packed
  DVE max tree, so neither engine is the wall.  relu(m+b) runs on ACT inside
  the stream as pooled columns land.  Head: per 512-target chunk, three
  single-shot [*,512] matmuls (multi-matmul PSUM accumulation groups fault on
  HW when banks are reused or sit at partition offset 64) combined via ACT
  copy + two DVE adds, relu+bias to a bf16 [128, 3328] output tile (target
  halves stacked on partitions), chunked DMA out.  Host unstacks + casts.

The (t,k) -> E column permutation (from the partition-stacking) is folded
into the host gather; fLT / output stay in plain target order.
"""

import os
import sys
import numpy as np
import ml_dtypes

for _p in ("/opt/trn_rl_repo", "/opt/pypackages"):
    if os.path.isdir(_p) and _p not in sys.path:
        sys.path.append(_p)

import concourse.bass as bass
import concourse.mybir as mybir
import concourse.tile as tile
from concourse import bacc

EPS = 1e-3
N_CORES = 8
F32 = mybir.dt.float32
BF16 = mybir.dt.bfloat16
NPBF16 = ml_dtypes.bfloat16

# problem dims (fixed by the task)
N_LAST, M1, M2, K = 50000, 200000, 100000, 16
C1, C2, CL, CG = 32, 64, 64, 64

NT = 6656                 # padded targets per core (52 * 128)
STEP_T = 128              # targets per PSUM step
NSTEP = NT // STEP_T      # 52
COLS = NT * K             # 106496 E columns per scale
STEP_C = STEP_T * K       # 2048 E columns per step
LOAD_STEPS = 4            # steps per E DMA load
LOAD_C = STEP_C * LOAD_STEPS
HALF_T = NT // 2          # 3328 targets per partition-half
ECH = C2 + C1             # 96 stacked channels in E


def _head_chunks():
    out, c0 = [], 0
    while c0 < HALF_T:
        w = min(512, HALF_T - c0)
        out.append((c0, w))
        c0 += w
    return out


def _emit(tc, io):
    nc = tc.nc

    with (
        tc.tile_pool(name="consts", bufs=1) as consts,
        tc.tile_pool(name="persist", bufs=1) as persist,
        tc.tile_pool(name="load", bufs=3) as load_pool,
    ):
        w2sb = consts.tile([C2, CG], BF16)
        w1pad = consts.tile([ECH, CG], BF16)
        wA0 = consts.tile([CL, CG], BF16)
        wB0 = consts.tile([CG, CG], BF16)
        wC0 = consts.tile([CG, CG], BF16)
        wBp = consts.tile([128, CG], BF16)
        wCp = consts.tile([128, CG], BF16)
        b1sb = consts.tile([128, 1], F32)
        b2sb = consts.tile([128, 1], F32)
        bosb = consts.tile([128, 1], F32)
        nc.scalar.dma_start(w2sb[:], io["w2f"].ap())
        nc.scalar.dma_start(w1pad[C2:ECH, :], io["w1f"].ap())
        nc.scalar.dma_start(wA0[:], io["wA"].ap())
        nc.scalar.dma_start(wB0[:], io["wB"].ap())
        nc.scalar.dma_start(wC0[:], io["wC"].ap())
        nc.scalar.dma_start(wBp[64:128, :], io["wB"].ap())
        nc.scalar.dma_start(wCp[64:128, :], io["wC"].ap())
        nc.scalar.dma_start(b1sb[:], io["b1"].ap())
        nc.scalar.dma_start(b2sb[:], io["b2"].ap())
        nc.scalar.dma_start(bosb[:], io["bo"].ap())

        flT = persist.tile([CL, NT], BF16)
        # (flT DMA deferred to mid-stream: its 0.85MB transfer otherwise
        # steals DMA-engine bandwidth from the critical first E loads)
        # pooled maxima, col order (g, b2, u); partition half h = target half
        mh1 = persist.tile([128, NSTEP, 2, 32], BF16)
        mh2 = persist.tile([128, NSTEP, 2, 32], BF16)
        # output, target halves stacked on partitions (ch c of target
        # h*HALF_T+j at partition h*64+c, col j); bf16, host casts to f32
        out_sb = persist.tile([128, HALF_T], BF16)
        rT1 = persist.tile([128, HALF_T], BF16)
        rT2 = persist.tile([128, HALF_T], BF16)

        e_ap = io["E"].ap()

        # Pooling runs in one of three modes so the reduce work spreads over
        # DVE, ACT and the otherwise-idle GpSimd: 0 = DVE tensor_reduce
        # straight from PSUM (1x microcode, input-bound); 1/2 = ACT copies
        # the PSUM group to bf16 SBUF, then DVE (2x packed-bf16) or GpSimd
        # runs a 4-level max tree.
        def pool_chunk(mode, ps, mh, i, zc_pool, tr_pool, tg):
            if mode == 0:
                zv = ps[:].rearrange("p b (u k) -> p b u k", k=K)
                nc.vector.tensor_reduce(
                    mh[:, i, :, :], zv[:],
                    axis=mybir.AxisListType.X,
                    op=mybir.AluOpType.max)
                return
            eng = nc.vector
            zc = zc_pool.tile([128, 2, 32, K], BF16, tag="zc" + tg)
            nc.scalar.copy(
                zc[:].rearrange("p b u k -> p (b u k)"),
                ps[:].rearrange("p b f -> p (b f)"))
            t8 = tr_pool.tile([128, 2, 32, 8], BF16, tag="t8" + tg)
            eng.tensor_max(t8[:], zc[:, :, :, 0:8], zc[:, :, :, 8:16])
            t4 = tr_pool.tile([128, 2, 32, 4], BF16, tag="t4" + tg)
            eng.tensor_max(t4[:], t8[:, :, :, 0:4], t8[:, :, :, 4:8])
            t2 = tr_pool.tile([128, 2, 32, 2], BF16, tag="t2" + tg)
            eng.tensor_max(t2[:], t4[:, :, :, 0:2], t4[:, :, :, 2:4])
            eng.tensor_max(mh[:, i, :, :], t2[:, :, :, 0], t2[:, :, :, 1])

        # emit each chunk's relu 2 steps after its last pooled column lands
        # so it never head-of-line-blocks the ACT queue (stalling the zc
        # copies stalls PSUM drain and collapses the PE p-state)
        relu_after, relu_tail = {}, []
        for (c0, w) in _head_chunks():
            key = (c0 + w - 1) // 64 + 2
            if key < NSTEP:
                relu_after.setdefault(key, []).append((c0, w))
            else:
                relu_tail.append((c0, w))

        with (
            tc.tile_pool(name="ps2", bufs=2, space="PSUM") as ps2_pool,
            tc.tile_pool(name="ps1", bufs=2, space="PSUM") as ps1_pool,
            tc.tile_pool(name="zc", bufs=3) as zc_pool,
            tc.tile_pool(name="trv", bufs=2) as trv_pool,
            tc.tile_pool(name="trg", bufs=2) as trg_pool,
        ):
            n_loads = COLS // LOAD_C
            for li in range(n_loads):
                et = load_pool.tile([ECH, LOAD_C], BF16, tag="et")
                if li == 0:
                    nc.sync.dma_start(et[:, 0:STEP_C], e_ap[:, 0:STEP_C])
                    nc.sync.dma_start(et[:, STEP_C:LOAD_C],
                                      e_ap[:, STEP_C:LOAD_C])
                else:
                    nc.sync.dma_start(
                        et[:], e_ap[:, li * LOAD_C:(li + 1) * LOAD_C])
                for j in range(LOAD_STEPS):
                    i = li * LOAD_STEPS + j
                    for (sc, (pool, w_ap, p0, p1, mh)) in enumerate((
                        (ps2_pool, w2sb[:], 0, C2, mh2),
                        (ps1_pool, w1pad[C2:ECH, :], C2, ECH, mh1),
                    )):
                        ps = pool.tile([128, 2, 512], F32,
                                       tag="ps" + ("s2", "s1")[sc])
                        for m in range(4):
                            h, b2 = m % 2, m // 2
                            nc.tensor.matmul(
                                ps[h * 64:(h + 1) * 64, b2, :],
                                lhsT=w_ap,
                                rhs=et[p0:p1,
                                       j * STEP_C + m * 512:
                                       j * STEP_C + (m + 1) * 512],
                                start=True, stop=True,
                            )
                        # ~30% direct PSUM reduce on DVE, ~70% via ACT copy
                        # + 2x-packed-bf16 DVE max tree (balances DVE ~94us
                        # vs ACT ~87us, both under the PE wall)
                        mode = 0 if (2 * i + sc) % 10 < 3 else 1
                        pool_chunk(mode, ps, mh, i, zc_pool, trv_pool, "v")
                    if i == min(20, NSTEP - 1):
                        nc.scalar.dma_start(flT[:], io["fLT"].ap())
                    # relu(m + b) for any head chunk whose pooled columns
                    # completed with this step — keeps the head tail short
                    for (c0, w) in relu_after.get(i, ()):
                        mh1f = mh1[:].rearrange("p g b u -> p (g b u)")
                        mh2f = mh2[:].rearrange("p g b u -> p (g b u)")
                        nc.scalar.activation(
                            rT1[:, c0:c0 + w], mh1f[:, c0:c0 + w],
                            mybir.ActivationFunctionType.Relu,
                            bias=b1sb[:, 0:1], scale=1.0)
                        nc.scalar.activation(
                            rT2[:, c0:c0 + w], mh2f[:, c0:c0 + w],
                            mybir.ActivationFunctionType.Relu,
                            bias=b2sb[:, 0:1], scale=1.0)

        # ---- head ----
        with (
            tc.tile_pool(name="hsum", bufs=2) as hsum_pool,
            tc.tile_pool(name="hps", bufs=2, space="PSUM") as hps_pool,
        ):
            for (c0, w) in relu_tail:
                mh1f = mh1[:].rearrange("p g b u -> p (g b u)")
                mh2f = mh2[:].rearrange("p g b u -> p (g b u)")
                nc.scalar.activation(
                    rT1[:, c0:c0 + w], mh1f[:, c0:c0 + w],
                    mybir.ActivationFunctionType.Relu,
                    bias=b1sb[:, 0:1], scale=1.0)
                nc.scalar.activation(
                    rT2[:, c0:c0 + w], mh2f[:, c0:c0 + w],
                    mybir.ActivationFunctionType.Relu,
                    bias=b2sb[:, 0:1], scale=1.0)
            # Multi-matmul PSUM accumulation groups fault on HW when their
            # bank is reused or sits at partition offset 64; single-shot
            # matmuls have neither problem (streaming phase proves both).
            # So: three single-shot matmuls into three banks, combined with
            # ACT copy + two DVE adds (each reading only one PSUM input).
            for (c0, w) in _head_chunks():
                poA = hps_pool.tile([128, 512], F32, tag="poA")
                poB = hps_pool.tile([128, 512], F32, tag="poB")
                poC = hps_pool.tile([128, 512], F32, tag="poC")
                for h in range(2):
                    sl = slice(h * 64, (h + 1) * 64)
                    nc.tensor.matmul(
                        poA[sl, :w], lhsT=wA0[:],
                        rhs=flT[:, h * HALF_T + c0:h * HALF_T + c0 + w],
                        start=True, stop=True)
                    nc.tensor.matmul(
                        poB[sl, :w],
                        lhsT=wB0[:] if h == 0 else wBp[64:128, :],
                        rhs=rT1[sl, c0:c0 + w],
                        start=True, stop=True)
                    nc.tensor.matmul(
                        poC[sl, :w],
                        lhsT=wC0[:] if h == 0 else wCp[64:128, :],
                        rhs=rT2[sl, c0:c0 + w],
                        start=True, stop=True)
                sA = hsum_pool.tile([128, 512], F32, tag="sA")
                nc.scalar.copy(sA[:, :w], poA[:, :w])
                sB = hsum_pool.tile([128, 512], F32, tag="sB")
                nc.vector.tensor_add(sB[:, :w], sA[:, :w], poB[:, :w])
                sC = hsum_pool.tile([128, 512], F32, tag="sC")
                nc.vector.tensor_add(sC[:, :w], sB[:, :w], poC[:, :w])
                nc.scalar.activation(
                    out_sb[:, c0:c0 + w], sC[:, :w],
                    mybir.ActivationFunctionType.Relu,
                    bias=bosb[:, 0:1], scale=1.0)
                nc.sync.dma_start(io["oT"].ap()[:, c0:c0 + w],
                                  out_sb[:, c0:c0 + w])


def build():
    nc = bacc.Bacc(
        "TRN2",
        target_bir_lowering=False,
        debug=False,
        enable_asserts=False,
        num_devices=N_CORES,
        num_swdge_queues=4,
    )
    io = {
        "E": nc.dram_tensor("E", [ECH, COLS], BF16, kind="ExternalInput"),
        "fLT": nc.dram_tensor("fLT", [CL, NT], BF16, kind="ExternalInput"),
        "w1f": nc.dram_tensor("w1f", [C1, CG], BF16, kind="ExternalInput"),
        "w2f": nc.dram_tensor("w2f", [C2, CG], BF16, kind="ExternalInput"),
        "wA": nc.dram_tensor("wA", [CL, CG], BF16, kind="ExternalInput"),
        "wB": nc.dram_tensor("wB", [CG, CG], BF16, kind="ExternalInput"),
        "wC": nc.dram_tensor("wC", [CG, CG], BF16, kind="ExternalInput"),
        "b1": nc.dram_tensor("b1", [128, 1], F32, kind="ExternalInput"),
        "b2": nc.dram_tensor("b2", [128, 1], F32, kind="ExternalInput"),
        "bo": nc.dram_tensor("bo", [128, 1], F32, kind="ExternalInput"),
        "oT": nc.dram_tensor("oT", [128, NT // 2], BF16, kind="ExternalOutput"),
    }
    with tile.TileContext(nc) as tc:
        _emit(tc, io)
    nc.compile()
    return nc


def host_prep_weights(Wg1, bn_g1, Wg2, bn_g2, Wout, bn_out):
    def bn_fold(p):
        g, b, m, v = p[0], p[1], p[2], p[3]
        s = g / np.sqrt(v + EPS)
        return s, b - m * s

    s1, t1 = bn_fold(bn_g1.astype(np.float64))
    s2, t2 = bn_fold(bn_g2.astype(np.float64))
    so, to = bn_fold(bn_out.astype(np.float64))
    sg1 = np.where(s1 >= 0, 1.0, -1.0)
    sg2 = np.where(s2 >= 0, 1.0, -1.0)
    a1, a2 = np.abs(s1), np.abs(s2)

    cl = Wout.shape[0] - 2 * CG
    Wo = Wout.astype(np.float64)
    b1p = (t1 / a1).astype(np.float32).reshape(CG, 1)
    b2p = (t2 / a2).astype(np.float32).reshape(CG, 1)
    return dict(
        w1f=(Wg1.astype(np.float64) * sg1[None, :]).astype(NPBF16),
        w2f=(Wg2.astype(np.float64) * sg2[None, :]).astype(NPBF16),
        wA=(Wo[:cl] * so[None, :]).astype(NPBF16),
        wB=(a1[:, None] * Wo[cl:cl + CG] * so[None, :]).astype(NPBF16),
        wC=(a2[:, None] * Wo[cl + CG:] * so[None, :]).astype(NPBF16),
        b1=np.concatenate([b1p, b1p], axis=0),
        b2=np.concatenate([b2p, b2p], axis=0),
        bo=np.concatenate([to, to]).astype(np.float32).reshape(128, 1),
    )


def _col_maps():
    """E column c -> (target, k).  Device consumption order per 2048-col
    step i: matmul m = 2*b2 + h covers cols i*2048 + m*512 + u*16 + k and
    lands at psum (partition h*64+ch, bank b2, u); pooled target index is
    t = h*HALF_T + i*64 + b2*32 + u."""
    c = np.arange(COLS)
    g = c // STEP_C
    m = (c // 512) % 4
    u = (c // K) % 32
    k = c % K
    t = (m % 2) * HALF_T + g * 64 + (m // 2) * 32 + u
    return t.astype(np.int64), k.astype(np.int64)


_T_OF_C, _K_OF_C = _col_maps()


def _host_prep(feat_s1, feat_s2, feat_last, Wg1, bn_g1, Wg2, bn_g2,
               Wout, bn_out, idx_s1, idx_s2):
    common = host_prep_weights(Wg1, bn_g1, Wg2, bn_g2, Wout, bn_out)

    f1b = feat_s1.astype(NPBF16)
    f2b = feat_s2.astype(NPBF16)
    n = feat_last.shape[0]
    n_shard = n // N_CORES

    in_maps = []
    for core in range(N_CORES):
        lo, hi = core * n_shard, (core + 1) * n_shard
        i1 = np.zeros((NT, K), np.int64)
        i1[:n_shard] = idx_s1[lo:hi]
        i2 = np.zeros((NT, K), np.int64)
        i2[:n_shard] = idx_s2[lo:hi]
        E = np.empty((ECH, COLS), NPBF16)
        E[:C2] = f2b[i2[_T_OF_C, _K_OF_C]].T
        E[C2:] = f1b[i1[_T_OF_C, _K_OF_C]].T
        flT = np.zeros((NT, CL), np.float32)
        flT[:n_shard] = feat_last[lo:hi]
        in_maps.append(dict(
            common, E=E, fLT=np.ascontiguousarray(flT.T.astype(NPBF16))))
    return in_maps, n_shard


_BUILD_CACHE = {}


def _ensure_profile_hook():
    """This image's ``antenv`` lacks ``axon_hooks``; concourse's trace=True
    path imports it unconditionally. Provide the module and install the
    ctypes NTFF hook against libaxon_pjrt.so (mirrors trn_boot.py)."""
    import types
    import ctypes
    import contextlib

    try:
        from antenv.axon_hooks import get_axon_ntff_profile_hook  # noqa: F401
        return
    except ImportError:
        pass

    mod = types.ModuleType("antenv.axon_hooks")
    mod._hook = None
    mod.set_axon_ntff_profile_hook = lambda h: setattr(mod, "_hook", h)
    mod.get_axon_ntff_profile_hook = lambda: mod._hook
    sys.modules["antenv.axon_hooks"] = mod
    import antenv
    antenv.axon_hooks = mod

    so_path = "/opt/axon/libaxon_pjrt.so"
    if not os.path.exists(so_path):
        return
    lib = ctypes.CDLL(so_path)
    if not hasattr(lib, "axon_start_nrt_profile"):
        return
    lib.axon_start_nrt_profile.argtypes = [
        ctypes.POINTER(ctypes.c_int64), ctypes.c_size_t,
    ]
    lib.axon_start_nrt_profile.restype = ctypes.c_int64
    lib.axon_stop_nrt_profile.argtypes = [ctypes.c_char_p]
    lib.axon_stop_nrt_profile.restype = ctypes.c_int64

    @contextlib.contextmanager
    def _hook(output_dir, device_ids):
        import jax
        jax.devices()
        if device_ids:
            ids = (ctypes.c_int64 * len(device_ids))(*device_ids)
            rc = lib.axon_start_nrt_profile(ids, len(device_ids))
        else:
            rc = lib.axon_start_nrt_profile(None, 0)
        if rc != 0:
            raise RuntimeError(f"axon_start_nrt_profile rc={rc}")
        try:
            yield
        finally:
            nf = lib.axon_stop_nrt_profile(str(output_dir).encode())
            print(f"profile: {nf} file(s) written to {output_dir}",
                  file=sys.stderr)

    mod.set_axon_ntff_profile_hook(_hook)


def kernel(**inputs):
    from concourse import bass_utils
    from concourse.bass_interp import get_hw_module

    in_maps, n_shard = _host_prep(**inputs)
    if "nc" not in _BUILD_CACHE:
        _BUILD_CACHE["nc"] = build()
    nc = _BUILD_CACHE["nc"]

    old_m = nc.m
    nc.m = get_hw_module(nc.m)
    try:
        trace = os.environ.get("BIGNN_TRACE", "0") == "1"
        if trace:
            _ensure_profile_hook()
        res = bass_utils.run_bass_kernel_spmd(
            nc, in_maps, core_ids=list(range(N_CORES)),
            trace=trace,
            trace_cores=list(range(N_CORES)) if trace else None,
        )
    finally:
        nc.m = old_m

    kernel.last_results = res
    n = inputs["feat_last"].shape[0]
    out = np.empty((n, CG), np.float32)
    for c in range(N_CORES):
        oT = res.results[c]["oT"].astype(np.float32)
        full = np.concatenate([oT[:CG, :].T, oT[CG:, :].T], axis=0)
        out[c * n_shard:(c + 1) * n_shard] = full[:n_shard]
    return out


# revision 29
# speedup vs baseline: 1.0582x; 1.0582x over previous
"""Trainium2 Bass kernel for nn_BiGNN (gnn_message_passing).

Math: p_i = max_k relu(bn_i(feat_i[idx_i] @ Wg_i)); out = relu(bn_o(cat @ Wout)).
BN is folded on the host (sign into Wg columns, |scale| into head weights):
  z_i = feat_i @ (Wg_i * sign(s_i)); m_i = max_k z_i[idx_i]
  out = relu(featL @ WA + relu(m1+b1') @ WB + relu(m2+b2') @ WC + bo)

Strategy (8 cores, data-parallel over the 50k target voxels, 6656 padded
targets per core): the neighbor gather is done ON THE HOST — the device
receives per-core pre-gathered, bf16, channel-major "expanded" feature
tables E[(s2 ch 0..63 | s1 ch 0..31), col] where col encodes (t, k) in the
exact order the device consumes.  No dma_gather, no index tables, no
transposes on device:

  per 2048-col step and scale: 4 matmuls with the (BN-folded) Wg STATIONARY
  stream E columns into one 2-bank PSUM group [128, 2x512] (two 64-channel
  target-halves stacked on partitions so all 128 DVE lanes work).  Pooling
  over k=16 column groups is split ~30/70 between a direct DVE tensor_reduce
  from PSUM (1x microcode) and an ACT PSUM->bf16 copy followed by a 2x-packed
  DVE max tree, so neither engine is the wall.  relu(m+b) runs on ACT inside
  the stream as pooled columns land.  Head: per 512-target chunk, three
  single-shot [*,512] matmuls (multi-matmul PSUM accumulation groups fault on
  HW when banks are reused or sit at partition offset 64) combined via ACT
  copy + two DVE adds, relu+bias to a bf16 [128, 3328] output tile (target
  halves stacked on partitions), chunked DMA out.  Host unstacks + casts.

The (t,k) -> E column permutation (from the partition-stacking) is folded
into the host gather; fLT / output stay in plain target order.
"""

import os
import sys
import numpy as np
import ml_dtypes

for _p in ("/opt/trn_rl_repo", "/opt/pypackages"):
    if os.path.isdir(_p) and _p not in sys.path:
        sys.path.append(_p)

import concourse.bass as bass
import concourse.mybir as mybir
import concourse.tile as tile
from concourse import bacc

EPS = 1e-3
N_CORES = 8
F32 = mybir.dt.float32
BF16 = mybir.dt.bfloat16
NPBF16 = ml_dtypes.bfloat16

# problem dims (fixed by the task)
N_LAST, M1, M2, K = 50000, 200000, 100000, 16
C1, C2, CL, CG = 32, 64, 64, 64

NT = 6656                 # padded targets per core (52 * 128)
STEP_T = 128              # targets per PSUM step
NSTEP = NT // STEP_T      # 52
COLS = NT * K             # 106496 E columns per scale
STEP_C = STEP_T * K       # 2048 E columns per step
LOAD_STEPS = 2            # steps per E DMA load
LOAD_C = STEP_C * LOAD_STEPS
HALF_T = NT // 2          # 3328 targets per partition-half
ECH = C2 + C1             # 96 stacked channels in E


def _head_chunks():
    out, c0 = [], 0
    while c0 < HALF_T:
        w = min(512, HALF_T - c0)
        out.append((c0, w))
        c0 += w
    return out


def _emit(tc, io):
    nc = tc.nc

    with (
        tc.tile_pool(name="consts", bufs=1) as consts,
        tc.tile_pool(name="persist", bufs=1) as persist,
        tc.tile_pool(name="load", bufs=3) as load_pool,
        tc.tile_pool(name="loadb", bufs=3) as loadb_pool,
    ):
        w2sb = consts.tile([C2, CG], BF16)
        w1pad = consts.tile([ECH, CG], BF16)
        wA0 = consts.tile([CL, CG], BF16)
        wB0 = consts.tile([CG, CG], BF16)
        wC0 = consts.tile([CG, CG], BF16)
        wBp = consts.tile([128, CG], BF16)
        wCp = consts.tile([128, CG], BF16)
        b1sb = consts.tile([128, 1], F32)
        b2sb = consts.tile([128, 1], F32)
        bosb = consts.tile([128, 1], F32)
        nc.scalar.dma_start(w2sb[:], io["w2f"].ap())
        nc.scalar.dma_start(w1pad[C2:ECH, :], io["w1f"].ap())
        nc.scalar.dma_start(wA0[:], io["wA"].ap())
        nc.scalar.dma_start(wB0[:], io["wB"].ap())
        nc.scalar.dma_start(wC0[:], io["wC"].ap())
        nc.scalar.dma_start(wBp[64:128, :], io["wB"].ap())
        nc.scalar.dma_start(wCp[64:128, :], io["wC"].ap())
        nc.scalar.dma_start(b1sb[:], io["b1"].ap())
        nc.scalar.dma_start(b2sb[:], io["b2"].ap())
        nc.scalar.dma_start(bosb[:], io["bo"].ap())

        flT = persist.tile([CL, NT], BF16)
        # (flT DMA deferred to mid-stream: its 0.85MB transfer otherwise
        # steals DMA-engine bandwidth from the critical first E loads)
        # pooled maxima, col order (g, b2, u); partition half h = target half
        mh1 = persist.tile([128, NSTEP, 2, 32], BF16)
        mh2 = persist.tile([128, NSTEP, 2, 32], BF16)
        # output, target halves stacked on partitions (ch c of target
        # h*HALF_T+j at partition h*64+c, col j); bf16, host casts to f32
        out_sb = persist.tile([128, HALF_T], BF16)
        rT1 = persist.tile([128, HALF_T], BF16)
        rT2 = persist.tile([128, HALF_T], BF16)

        e_ap = io["E"].ap()

        # Pooling runs in one of three modes so the reduce work spreads over
        # DVE, ACT and the otherwise-idle GpSimd: 0 = DVE tensor_reduce
        # straight from PSUM (1x microcode, input-bound); 1/2 = ACT copies
        # the PSUM group to bf16 SBUF, then DVE (2x packed-bf16) or GpSimd
        # runs a 4-level max tree.
        def pool_chunk(mode, ps, mh, i, zc_pool, tr_pool, tg):
            if mode == 0:
                zv = ps[:].rearrange("p b (u k) -> p b u k", k=K)
                nc.vector.tensor_reduce(
                    mh[:, i, :, :], zv[:],
                    axis=mybir.AxisListType.X,
                    op=mybir.AluOpType.max)
                return
            eng = nc.vector
            zc = zc_pool.tile([128, 2, 32, K], BF16, tag="zc" + tg)
            nc.scalar.copy(
                zc[:].rearrange("p b u k -> p (b u k)"),
                ps[:].rearrange("p b f -> p (b f)"))
            t8 = tr_pool.tile([128, 2, 32, 8], BF16, tag="t8" + tg)
            eng.tensor_max(t8[:], zc[:, :, :, 0:8], zc[:, :, :, 8:16])
            t4 = tr_pool.tile([128, 2, 32, 4], BF16, tag="t4" + tg)
            eng.tensor_max(t4[:], t8[:, :, :, 0:4], t8[:, :, :, 4:8])
            t2 = tr_pool.tile([128, 2, 32, 2], BF16, tag="t2" + tg)
            eng.tensor_max(t2[:], t4[:, :, :, 0:2], t4[:, :, :, 2:4])
            eng.tensor_max(mh[:, i, :, :], t2[:, :, :, 0], t2[:, :, :, 1])

        # emit each chunk's relu 2 steps after its last pooled column lands
        # so it never head-of-line-blocks the ACT queue (stalling the zc
        # copies stalls PSUM drain and collapses the PE p-state)
        relu_after, relu_tail = {}, []
        for (c0, w) in _head_chunks():
            key = (c0 + w - 1) // 64 + 2
            if key < NSTEP:
                relu_after.setdefault(key, []).append((c0, w))
            else:
                relu_tail.append((c0, w))

        with (
            tc.tile_pool(name="ps2", bufs=2, space="PSUM") as ps2_pool,
            tc.tile_pool(name="ps1", bufs=2, space="PSUM") as ps1_pool,
            tc.tile_pool(name="zc", bufs=3) as zc_pool,
            tc.tile_pool(name="trv", bufs=2) as trv_pool,
            tc.tile_pool(name="trg", bufs=2) as trg_pool,
        ):
            n_loads = COLS // LOAD_C
            for li in range(n_loads):
                # alternate the two HWDGE queues (SP / ACT) so load
                # transfers, which drain FIFO per queue, deliver at 2x the
                # single-queue cadence; per-queue pools are deep enough that
                # a load's tile-free wait is satisfied long before it could
                # head-of-line-block the ACT queue
                if li % 2 == 0:
                    et = load_pool.tile([ECH, LOAD_C], BF16, tag="et")
                    q = nc.sync
                else:
                    et = loadb_pool.tile([ECH, LOAD_C], BF16, tag="etb")
                    q = nc.scalar
                if li == 0:
                    q.dma_start(et[:, 0:STEP_C], e_ap[:, 0:STEP_C])
                    q.dma_start(et[:, STEP_C:LOAD_C], e_ap[:, STEP_C:LOAD_C])
                else:
                    q.dma_start(
                        et[:], e_ap[:, li * LOAD_C:(li + 1) * LOAD_C])
                for j in range(LOAD_STEPS):
                    i = li * LOAD_STEPS + j
                    for (sc, (pool, w_ap, p0, p1, mh)) in enumerate((
                        (ps2_pool, w2sb[:], 0, C2, mh2),
                        (ps1_pool, w1pad[C2:ECH, :], C2, ECH, mh1),
                    )):
                        ps = pool.tile([128, 2, 512], F32,
                                       tag="ps" + ("s2", "s1")[sc])
                        for m in range(4):
                            h, b2 = m % 2, m // 2
                            nc.tensor.matmul(
                                ps[h * 64:(h + 1) * 64, b2, :],
                                lhsT=w_ap,
                                rhs=et[p0:p1,
                                       j * STEP_C + m * 512:
                                       j * STEP_C + (m + 1) * 512],
                                start=True, stop=True,
                            )
                        # ~30% direct PSUM reduce on DVE, ~70% via ACT copy
                        # + 2x-packed-bf16 DVE max tree (balances DVE ~94us
                        # vs ACT ~87us, both under the PE wall)
                        mode = 0 if (2 * i + sc) % 10 < 3 else 1
                        pool_chunk(mode, ps, mh, i, zc_pool, trv_pool, "v")
                    if i == min(20, NSTEP - 1):
                        nc.scalar.dma_start(flT[:], io["fLT"].ap())
                    # relu(m + b) for any head chunk whose pooled columns
                    # completed with this step — keeps the head tail short
                    for (c0, w) in relu_after.get(i, ()):
                        mh1f = mh1[:].rearrange("p g b u -> p (g b u)")
                        mh2f = mh2[:].rearrange("p g b u -> p (g b u)")
                        nc.scalar.activation(
                            rT1[:, c0:c0 + w], mh1f[:, c0:c0 + w],
                            mybir.ActivationFunctionType.Relu,
                            bias=b1sb[:, 0:1], scale=1.0)
                        nc.scalar.activation(
                            rT2[:, c0:c0 + w], mh2f[:, c0:c0 + w],
                            mybir.ActivationFunctionType.Relu,
                            bias=b2sb[:, 0:1], scale=1.0)

        # ---- head ----
        with (
            tc.tile_pool(name="hsum", bufs=2) as hsum_pool,
            tc.tile_pool(name="hps", bufs=2, space="PSUM") as hps_pool,
        ):
            for (c0, w) in relu_tail:
                mh1f = mh1[:].rearrange("p g b u -> p (g b u)")
                mh2f = mh2[:].rearrange("p g b u -> p (g b u)")
                nc.scalar.activation(
                    rT1[:, c0:c0 + w], mh1f[:, c0:c0 + w],
                    mybir.ActivationFunctionType.Relu,
                    bias=b1sb[:, 0:1], scale=1.0)
                nc.scalar.activation(
                    rT2[:, c0:c0 + w], mh2f[:, c0:c0 + w],
                    mybir.ActivationFunctionType.Relu,
                    bias=b2sb[:, 0:1], scale=1.0)
            # Multi-matmul PSUM accumulation groups fault on HW when their
            # bank is reused or sits at partition offset 64; single-shot
            # matmuls have neither problem (streaming phase proves both).
            # So: three single-shot matmuls into three banks, combined with
            # ACT copy + two DVE adds (each reading only one PSUM input).
            for (c0, w) in _head_chunks():
                poA = hps_pool.tile([128, 512], F32, tag="poA")
                poB = hps_pool.tile([128, 512], F32, tag="poB")
                poC = hps_pool.tile([128, 512], F32, tag="poC")
                for h in range(2):
                    sl = slice(h * 64, (h + 1) * 64)
                    nc.tensor.matmul(
                        poA[sl, :w], lhsT=wA0[:],
                        rhs=flT[:, h * HALF_T + c0:h * HALF_T + c0 + w],
                        start=True, stop=True)
                    nc.tensor.matmul(
                        poB[sl, :w],
                        lhsT=wB0[:] if h == 0 else wBp[64:128, :],
                        rhs=rT1[sl, c0:c0 + w],
                        start=True, stop=True)
                    nc.tensor.matmul(
                        poC[sl, :w],
                        lhsT=wC0[:] if h == 0 else wCp[64:128, :],
                        rhs=rT2[sl, c0:c0 + w],
                        start=True, stop=True)
                sA = hsum_pool.tile([128, 512], F32, tag="sA")
                nc.scalar.copy(sA[:, :w], poA[:, :w])
                sB = hsum_pool.tile([128, 512], F32, tag="sB")
                nc.vector.tensor_add(sB[:, :w], sA[:, :w], poB[:, :w])
                sC = hsum_pool.tile([128, 512], F32, tag="sC")
                nc.vector.tensor_add(sC[:, :w], sB[:, :w], poC[:, :w])
                nc.scalar.activation(
                    out_sb[:, c0:c0 + w], sC[:, :w],
                    mybir.ActivationFunctionType.Relu,
                    bias=bosb[:, 0:1], scale=1.0)
                nc.sync.dma_start(io["oT"].ap()[:, c0:c0 + w],
                                  out_sb[:, c0:c0 + w])


def build():
    nc = bacc.Bacc(
        "TRN2",
        target_bir_lowering=False,
        debug=False,
        enable_asserts=False,
        num_devices=N_CORES,
        num_swdge_queues=4,
    )
    io = {
        "E": nc.dram_tensor("E", [ECH, COLS], BF16, kind="ExternalInput"),
        "fLT": nc.dram_tensor("fLT", [CL, NT], BF16, kind="ExternalInput"),
        "w1f": nc.dram_tensor("w1f", [C1, CG], BF16, kind="ExternalInput"),
        "w2f": nc.dram_tensor("w2f", [C2, CG], BF16, kind="ExternalInput"),
        "wA": nc.dram_tensor("wA", [CL, CG], BF16, kind="ExternalInput"),
        "wB": nc.dram_tensor("wB", [CG, CG], BF16, kind="ExternalInput"),
        "wC": nc.dram_tensor("wC", [CG, CG], BF16, kind="ExternalInput"),
        "b1": nc.dram_tensor("b1", [128, 1], F32, kind="ExternalInput"),
        "b2": nc.dram_tensor("b2", [128, 1], F32, kind="ExternalInput"),
        "bo": nc.dram_tensor("bo", [128, 1], F32, kind="ExternalInput"),
        "oT": nc.dram_tensor("oT", [128, NT // 2], BF16, kind="ExternalOutput"),
    }
    with tile.TileContext(nc) as tc:
        _emit(tc, io)
    nc.compile()
    return nc


def host_prep_weights(Wg1, bn_g1, Wg2, bn_g2, Wout, bn_out):
    def bn_fold(p):
        g, b, m, v = p[0], p[1], p[2], p[3]
        s = g / np.sqrt(v + EPS)
        return s, b - m * s

    s1, t1 = bn_fold(bn_g1.astype(np.float64))
    s2, t2 = bn_fold(bn_g2.astype(np.float64))
    so, to = bn_fold(bn_out.astype(np.float64))
    sg1 = np.where(s1 >= 0, 1.0, -1.0)
    sg2 = np.where(s2 >= 0, 1.0, -1.0)
    a1, a2 = np.abs(s1), np.abs(s2)

    cl = Wout.shape[0] - 2 * CG
    Wo = Wout.astype(np.float64)
    b1p = (t1 / a1).astype(np.float32).reshape(CG, 1)
    b2p = (t2 / a2).astype(np.float32).reshape(CG, 1)
    return dict(
        w1f=(Wg1.astype(np.float64) * sg1[None, :]).astype(NPBF16),
        w2f=(Wg2.astype(np.float64) * sg2[None, :]).astype(NPBF16),
        wA=(Wo[:cl] * so[None, :]).astype(NPBF16),
        wB=(a1[:, None] * Wo[cl:cl + CG] * so[None, :]).astype(NPBF16),
        wC=(a2[:, None] * Wo[cl + CG:] * so[None, :]).astype(NPBF16),
        b1=np.concatenate([b1p, b1p], axis=0),
        b2=np.concatenate([b2p, b2p], axis=0),
        bo=np.concatenate([to, to]).astype(np.float32).reshape(128, 1),
    )


def _col_maps():
    """E column c -> (target, k).  Device consumption order per 2048-col
    step i: matmul m = 2*b2 + h covers cols i*2048 + m*512 + u*16 + k and
    lands at psum (partition h*64+ch, bank b2, u); pooled target index is
    t = h*HALF_T + i*64 + b2*32 + u."""
    c = np.arange(COLS)
    g = c // STEP_C
    m = (c // 512) % 4
    u = (c // K) % 32
    k = c % K
    t = (m % 2) * HALF_T + g * 64 + (m // 2) * 32 + u
    return t.astype(np.int64), k.astype(np.int64)


_T_OF_C, _K_OF_C = _col_maps()


def _host_prep(feat_s1, feat_s2, feat_last, Wg1, bn_g1, Wg2, bn_g2,
               Wout, bn_out, idx_s1, idx_s2):
    common = host_prep_weights(Wg1, bn_g1, Wg2, bn_g2, Wout, bn_out)

    f1b = feat_s1.astype(NPBF16)
    f2b = feat_s2.astype(NPBF16)
    n = feat_last.shape[0]
    n_shard = n // N_CORES

    in_maps = []
    for core in range(N_CORES):
        lo, hi = core * n_shard, (core + 1) * n_shard
        i1 = np.zeros((NT, K), np.int64)
        i1[:n_shard] = idx_s1[lo:hi]
        i2 = np.zeros((NT, K), np.int64)
        i2[:n_shard] = idx_s2[lo:hi]
        E = np.empty((ECH, COLS), NPBF16)
        E[:C2] = f2b[i2[_T_OF_C, _K_OF_C]].T
        E[C2:] = f1b[i1[_T_OF_C, _K_OF_C]].T
        flT = np.zeros((NT, CL), np.float32)
        flT[:n_shard] = feat_last[lo:hi]
        in_maps.append(dict(
            common, E=E, fLT=np.ascontiguousarray(flT.T.astype(NPBF16))))
    return in_maps, n_shard


_BUILD_CACHE = {}


def _ensure_profile_hook():
    """This image's ``antenv`` lacks ``axon_hooks``; concourse's trace=True
    path imports it unconditionally. Provide the module and install the
    ctypes NTFF hook against libaxon_pjrt.so (mirrors trn_boot.py)."""
    import types
    import ctypes
    import contextlib

    try:
        from antenv.axon_hooks import get_axon_ntff_profile_hook  # noqa: F401
        return
    except ImportError:
        pass

    mod = types.ModuleType("antenv.axon_hooks")
    mod._hook = None
    mod.set_axon_ntff_profile_hook = lambda h: setattr(mod, "_hook", h)
    mod.get_axon_ntff_profile_hook = lambda: mod._hook
    sys.modules["antenv.axon_hooks"] = mod
    import antenv
    antenv.axon_hooks = mod

    so_path = "/opt/axon/libaxon_pjrt.so"
    if not os.path.exists(so_path):
        return
    lib = ctypes.CDLL(so_path)
    if not hasattr(lib, "axon_start_nrt_profile"):
        return
    lib.axon_start_nrt_profile.argtypes = [
        ctypes.POINTER(ctypes.c_int64), ctypes.c_size_t,
    ]
    lib.axon_start_nrt_profile.restype = ctypes.c_int64
    lib.axon_stop_nrt_profile.argtypes = [ctypes.c_char_p]
    lib.axon_stop_nrt_profile.restype = ctypes.c_int64

    @contextlib.contextmanager
    def _hook(output_dir, device_ids):
        import jax
        jax.devices()
        if device_ids:
            ids = (ctypes.c_int64 * len(device_ids))(*device_ids)
            rc = lib.axon_start_nrt_profile(ids, len(device_ids))
        else:
            rc = lib.axon_start_nrt_profile(None, 0)
        if rc != 0:
            raise RuntimeError(f"axon_start_nrt_profile rc={rc}")
        try:
            yield
        finally:
            nf = lib.axon_stop_nrt_profile(str(output_dir).encode())
            print(f"profile: {nf} file(s) written to {output_dir}",
                  file=sys.stderr)

    mod.set_axon_ntff_profile_hook(_hook)


def kernel(**inputs):
    from concourse import bass_utils
    from concourse.bass_interp import get_hw_module

    in_maps, n_shard = _host_prep(**inputs)
    if "nc" not in _BUILD_CACHE:
        _BUILD_CACHE["nc"] = build()
    nc = _BUILD_CACHE["nc"]

    old_m = nc.m
    nc.m = get_hw_module(nc.m)
    try:
        trace = os.environ.get("BIGNN_TRACE", "0") == "1"
        if trace:
            _ensure_profile_hook()
        res = bass_utils.run_bass_kernel_spmd(
            nc, in_maps, core_ids=list(range(N_CORES)),
            trace=trace,
            trace_cores=list(range(N_CORES)) if trace else None,
        )
    finally:
        nc.m = old_m

    kernel.last_results = res
    n = inputs["feat_last"].shape[0]
    out = np.empty((n, CG), np.float32)
    for c in range(N_CORES):
        oT = res.results[c]["oT"].astype(np.float32)
        full = np.concatenate([oT[:CG, :].T, oT[CG:, :].T], axis=0)
        out[c * n_shard:(c + 1) * n_shard] = full[:n_shard]
    return out


# revision 30
# speedup vs baseline: 1.2413x; 1.1731x over previous
"""Trainium2 Bass kernel for nn_BiGNN (gnn_message_passing).

Math: p_i = max_k relu(bn_i(feat_i[idx_i] @ Wg_i)); out = relu(bn_o(cat @ Wout)).
BN is folded on the host (sign into Wg columns, |scale| into head weights):
  z_i = feat_i @ (Wg_i * sign(s_i)); m_i = max_k z_i[idx_i]
  out = relu(featL @ WA + relu(m1+b1') @ WB + relu(m2+b2') @ WC + bo)

Strategy (8 cores, data-parallel over the 50k target voxels, 6656 padded
targets per core): the neighbor gather is done ON THE HOST — the device
receives per-core pre-gathered, bf16, channel-major "expanded" feature
tables E[(s2 ch 0..63 | s1 ch 0..31), col] where col encodes (t, k) in the
exact order the device consumes.  No dma_gather, no index tables, no
transposes on device:

  per 2048-col step and scale: 4 matmuls with the (BN-folded) Wg STATIONARY
  stream E columns into one 2-bank PSUM group [128, 2x512] (two 64-channel
  target-halves stacked on partitions so all 128 DVE lanes work).  Pooling
  over k=16 column groups is split ~30/70 between a direct DVE tensor_reduce
  from PSUM (1x microcode) and an ACT PSUM->bf16 copy followed by a 2x-packed
  DVE max tree, so neither engine is the wall.  relu(m+b) runs on ACT inside
  the stream as pooled columns land.  Head: per 512-target chunk, three
  single-shot [*,512] matmuls (multi-matmul PSUM accumulation groups fault on
  HW when banks are reused or sit at partition offset 64) combined via ACT
  copy + two DVE adds, relu+bias to a bf16 [128, 3328] output tile (target
  halves stacked on partitions), chunked DMA out.  Host unstacks + casts.

The (t,k) -> E column permutation (from the partition-stacking) is folded
into the host gather; fLT / output stay in plain target order.
"""

import os
import sys
import numpy as np
import ml_dtypes

for _p in ("/opt/trn_rl_repo", "/opt/pypackages"):
    if os.path.isdir(_p) and _p not in sys.path:
        sys.path.append(_p)

import concourse.bass as bass
import concourse.mybir as mybir
import concourse.tile as tile
from concourse import bacc

EPS = 1e-3
N_CORES = 8
F32 = mybir.dt.float32
BF16 = mybir.dt.bfloat16
NPBF16 = ml_dtypes.bfloat16

# problem dims (fixed by the task)
N_LAST, M1, M2, K = 50000, 200000, 100000, 16
C1, C2, CL, CG = 32, 64, 64, 64

NT = 6656                 # padded targets per core (52 * 128)
STEP_T = 128              # targets per PSUM step
NSTEP = NT // STEP_T      # 52
COLS = NT * K             # 106496 E columns per scale
STEP_C = STEP_T * K       # 2048 E columns per step
LOAD_STEPS = 2            # steps per E DMA load
LOAD_C = STEP_C * LOAD_STEPS
HALF_T = NT // 2          # 3328 targets per partition-half
ECH = C2 + C1             # 96 stacked channels in E


def _head_chunks():
    out, c0 = [], 0
    while c0 < HALF_T:
        w = min(512, HALF_T - c0)
        out.append((c0, w))
        c0 += w
    return out


def _emit(tc, io):
    nc = tc.nc

    with (
        tc.tile_pool(name="consts", bufs=1) as consts,
        tc.tile_pool(name="persist", bufs=1) as persist,
        tc.tile_pool(name="load", bufs=4) as load_pool,
    ):
        w2sb = consts.tile([C2, CG], BF16)
        w1pad = consts.tile([ECH, CG], BF16)
        wA0 = consts.tile([CL, CG], BF16)
        wB0 = consts.tile([CG, CG], BF16)
        wC0 = consts.tile([CG, CG], BF16)
        wBp = consts.tile([128, CG], BF16)
        wCp = consts.tile([128, CG], BF16)
        b1sb = consts.tile([128, 1], F32)
        b2sb = consts.tile([128, 1], F32)
        bosb = consts.tile([128, 1], F32)
        nc.scalar.dma_start(w2sb[:], io["w2f"].ap())
        nc.scalar.dma_start(w1pad[C2:ECH, :], io["w1f"].ap())
        nc.scalar.dma_start(wA0[:], io["wA"].ap())
        nc.scalar.dma_start(wB0[:], io["wB"].ap())
        nc.scalar.dma_start(wC0[:], io["wC"].ap())
        nc.scalar.dma_start(wBp[64:128, :], io["wB"].ap())
        nc.scalar.dma_start(wCp[64:128, :], io["wC"].ap())
        nc.scalar.dma_start(b1sb[:], io["b1"].ap())
        nc.scalar.dma_start(b2sb[:], io["b2"].ap())
        nc.scalar.dma_start(bosb[:], io["bo"].ap())

        flT = persist.tile([CL, NT], BF16)
        # (flT DMA deferred to mid-stream: its 0.85MB transfer otherwise
        # steals DMA-engine bandwidth from the critical first E loads)
        # pooled maxima, col order (g, b2, u); partition half h = target half
        mh1 = persist.tile([128, NSTEP, 2, 32], BF16)
        mh2 = persist.tile([128, NSTEP, 2, 32], BF16)
        # output, target halves stacked on partitions (ch c of target
        # h*HALF_T+j at partition h*64+c, col j); bf16, host casts to f32
        out_sb = persist.tile([128, HALF_T], BF16)
        rT1 = persist.tile([128, HALF_T], BF16)
        rT2 = persist.tile([128, HALF_T], BF16)

        e_ap = io["E"].ap()

        # Pooling runs in one of three modes so the reduce work spreads over
        # DVE, ACT and the otherwise-idle GpSimd: 0 = DVE tensor_reduce
        # straight from PSUM (1x microcode, input-bound); 1/2 = ACT copies
        # the PSUM group to bf16 SBUF, then DVE (2x packed-bf16) or GpSimd
        # runs a 4-level max tree.
        def pool_chunk(mode, ps, mh, i, zc_pool, tr_pool, tg):
            if mode == 0:
                zv = ps[:].rearrange("p b (u k) -> p b u k", k=K)
                nc.vector.tensor_reduce(
                    mh[:, i, :, :], zv[:],
                    axis=mybir.AxisListType.X,
                    op=mybir.AluOpType.max)
                return
            eng = nc.vector
            zc = zc_pool.tile([128, 2, 32, K], BF16, tag="zc" + tg)
            nc.scalar.copy(
                zc[:].rearrange("p b u k -> p (b u k)"),
                ps[:].rearrange("p b f -> p (b f)"))
            t8 = tr_pool.tile([128, 2, 32, 8], BF16, tag="t8" + tg)
            eng.tensor_max(t8[:], zc[:, :, :, 0:8], zc[:, :, :, 8:16])
            t4 = tr_pool.tile([128, 2, 32, 4], BF16, tag="t4" + tg)
            eng.tensor_max(t4[:], t8[:, :, :, 0:4], t8[:, :, :, 4:8])
            t2 = tr_pool.tile([128, 2, 32, 2], BF16, tag="t2" + tg)
            eng.tensor_max(t2[:], t4[:, :, :, 0:2], t4[:, :, :, 2:4])
            eng.tensor_max(mh[:, i, :, :], t2[:, :, :, 0], t2[:, :, :, 1])

        # emit each chunk's relu 2 steps after its last pooled column lands
        # so it never head-of-line-blocks the ACT queue (stalling the zc
        # copies stalls PSUM drain and collapses the PE p-state)
        relu_after, relu_tail = {}, []
        for (c0, w) in _head_chunks():
            key = (c0 + w - 1) // 64 + 2
            if key < NSTEP:
                relu_after.setdefault(key, []).append((c0, w))
            else:
                relu_tail.append((c0, w))

        with (
            tc.tile_pool(name="ps2", bufs=2, space="PSUM") as ps2_pool,
            tc.tile_pool(name="ps1", bufs=2, space="PSUM") as ps1_pool,
            tc.tile_pool(name="zc", bufs=3) as zc_pool,
            tc.tile_pool(name="trv", bufs=2) as trv_pool,
            tc.tile_pool(name="trg", bufs=2) as trg_pool,
        ):
            n_loads = COLS // LOAD_C
            for li in range(n_loads):
                et = load_pool.tile([ECH, LOAD_C], BF16, tag="et")
                if li == 0:
                    nc.sync.dma_start(et[:, 0:STEP_C], e_ap[:, 0:STEP_C])
                    nc.sync.dma_start(et[:, STEP_C:LOAD_C],
                                      e_ap[:, STEP_C:LOAD_C])
                else:
                    nc.sync.dma_start(
                        et[:], e_ap[:, li * LOAD_C:(li + 1) * LOAD_C])
                for j in range(LOAD_STEPS):
                    i = li * LOAD_STEPS + j
                    for (sc, (pool, w_ap, p0, p1, mh)) in enumerate((
                        (ps2_pool, w2sb[:], 0, C2, mh2),
                        (ps1_pool, w1pad[C2:ECH, :], C2, ECH, mh1),
                    )):
                        ps = pool.tile([128, 2, 512], F32,
                                       tag="ps" + ("s2", "s1")[sc])
                        for m in range(4):
                            h, b2 = m % 2, m // 2
                            nc.tensor.matmul(
                                ps[h * 64:(h + 1) * 64, b2, :],
                                lhsT=w_ap,
                                rhs=et[p0:p1,
                                       j * STEP_C + m * 512:
                                       j * STEP_C + (m + 1) * 512],
                                start=True, stop=True,
                            )
                        # ~20% direct PSUM reduce on DVE, ~80% via ACT copy
                        # + 2x-packed-bf16 DVE max tree (balances DVE ~94us
                        # vs ACT ~87us, both under the PE wall)
                        mode = 0 if (2 * i + sc) % 10 < 2 else 1
                        pool_chunk(mode, ps, mh, i, zc_pool, trv_pool, "v")
                    if i == min(20, NSTEP - 1):
                        nc.scalar.dma_start(flT[:], io["fLT"].ap())
                    # relu(m + b) for any head chunk whose pooled columns
                    # completed with this step — keeps the head tail short
                    for (c0, w) in relu_after.get(i, ()):
                        mh1f = mh1[:].rearrange("p g b u -> p (g b u)")
                        mh2f = mh2[:].rearrange("p g b u -> p (g b u)")
                        nc.scalar.activation(
                            rT1[:, c0:c0 + w], mh1f[:, c0:c0 + w],
                            mybir.ActivationFunctionType.Relu,
                            bias=b1sb[:, 0:1], scale=1.0)
                        nc.scalar.activation(
                            rT2[:, c0:c0 + w], mh2f[:, c0:c0 + w],
                            mybir.ActivationFunctionType.Relu,
                            bias=b2sb[:, 0:1], scale=1.0)

        # ---- head ----
        with (
            tc.tile_pool(name="hsum", bufs=2) as hsum_pool,
            tc.tile_pool(name="hps", bufs=2, space="PSUM") as hps_pool,
        ):
            for (c0, w) in relu_tail:
                mh1f = mh1[:].rearrange("p g b u -> p (g b u)")
                mh2f = mh2[:].rearrange("p g b u -> p (g b u)")
                nc.scalar.activation(
                    rT1[:, c0:c0 + w], mh1f[:, c0:c0 + w],
                    mybir.ActivationFunctionType.Relu,
                    bias=b1sb[:, 0:1], scale=1.0)
                nc.scalar.activation(
                    rT2[:, c0:c0 + w], mh2f[:, c0:c0 + w],
                    mybir.ActivationFunctionType.Relu,
                    bias=b2sb[:, 0:1], scale=1.0)
            # Multi-matmul PSUM accumulation groups fault on HW when their
            # bank is reused or sits at partition offset 64; single-shot
            # matmuls have neither problem (streaming phase proves both).
            # So: three single-shot matmuls into three banks, combined with
            # ACT copy + two DVE adds (each reading only one PSUM input).
            for (c0, w) in _head_chunks():
                poA = hps_pool.tile([128, 512], F32, tag="poA")
                poB = hps_pool.tile([128, 512], F32, tag="poB")
                poC = hps_pool.tile([128, 512], F32, tag="poC")
                for h in range(2):
                    sl = slice(h * 64, (h + 1) * 64)
                    nc.tensor.matmul(
                        poA[sl, :w], lhsT=wA0[:],
                        rhs=flT[:, h * HALF_T + c0:h * HALF_T + c0 + w],
                        start=True, stop=True)
                    nc.tensor.matmul(
                        poB[sl, :w],
                        lhsT=wB0[:] if h == 0 else wBp[64:128, :],
                        rhs=rT1[sl, c0:c0 + w],
                        start=True, stop=True)
                    nc.tensor.matmul(
                        poC[sl, :w],
                        lhsT=wC0[:] if h == 0 else wCp[64:128, :],
                        rhs=rT2[sl, c0:c0 + w],
                        start=True, stop=True)
                sA = hsum_pool.tile([128, 512], F32, tag="sA")
                nc.scalar.copy(sA[:, :w], poA[:, :w])
                sB = hsum_pool.tile([128, 512], F32, tag="sB")
                nc.vector.tensor_add(sB[:, :w], sA[:, :w], poB[:, :w])
                sC = hsum_pool.tile([128, 512], F32, tag="sC")
                nc.vector.tensor_add(sC[:, :w], sB[:, :w], poC[:, :w])
                nc.scalar.activation(
                    out_sb[:, c0:c0 + w], sC[:, :w],
                    mybir.ActivationFunctionType.Relu,
                    bias=bosb[:, 0:1], scale=1.0)
                nc.sync.dma_start(io["oT"].ap()[:, c0:c0 + w],
                                  out_sb[:, c0:c0 + w])


def build():
    nc = bacc.Bacc(
        "TRN2",
        target_bir_lowering=False,
        debug=False,
        enable_asserts=False,
        num_devices=N_CORES,
        num_swdge_queues=4,
    )
    io = {
        "E": nc.dram_tensor("E", [ECH, COLS], BF16, kind="ExternalInput"),
        "fLT": nc.dram_tensor("fLT", [CL, NT], BF16, kind="ExternalInput"),
        "w1f": nc.dram_tensor("w1f", [C1, CG], BF16, kind="ExternalInput"),
        "w2f": nc.dram_tensor("w2f", [C2, CG], BF16, kind="ExternalInput"),
        "wA": nc.dram_tensor("wA", [CL, CG], BF16, kind="ExternalInput"),
        "wB": nc.dram_tensor("wB", [CG, CG], BF16, kind="ExternalInput"),
        "wC": nc.dram_tensor("wC", [CG, CG], BF16, kind="ExternalInput"),
        "b1": nc.dram_tensor("b1", [128, 1], F32, kind="ExternalInput"),
        "b2": nc.dram_tensor("b2", [128, 1], F32, kind="ExternalInput"),
        "bo": nc.dram_tensor("bo", [128, 1], F32, kind="ExternalInput"),
        "oT": nc.dram_tensor("oT", [128, NT // 2], BF16, kind="ExternalOutput"),
    }
    with tile.TileContext(nc) as tc:
        _emit(tc, io)
    nc.compile()
    return nc


def host_prep_weights(Wg1, bn_g1, Wg2, bn_g2, Wout, bn_out):
    def bn_fold(p):
        g, b, m, v = p[0], p[1], p[2], p[3]
        s = g / np.sqrt(v + EPS)
        return s, b - m * s

    s1, t1 = bn_fold(bn_g1.astype(np.float64))
    s2, t2 = bn_fold(bn_g2.astype(np.float64))
    so, to = bn_fold(bn_out.astype(np.float64))
    sg1 = np.where(s1 >= 0, 1.0, -1.0)
    sg2 = np.where(s2 >= 0, 1.0, -1.0)
    a1, a2 = np.abs(s1), np.abs(s2)

    cl = Wout.shape[0] - 2 * CG
    Wo = Wout.astype(np.float64)
    b1p = (t1 / a1).astype(np.float32).reshape(CG, 1)
    b2p = (t2 / a2).astype(np.float32).reshape(CG, 1)
    return dict(
        w1f=(Wg1.astype(np.float64) * sg1[None, :]).astype(NPBF16),
        w2f=(Wg2.astype(np.float64) * sg2[None, :]).astype(NPBF16),
        wA=(Wo[:cl] * so[None, :]).astype(NPBF16),
        wB=(a1[:, None] * Wo[cl:cl + CG] * so[None, :]).astype(NPBF16),
        wC=(a2[:, None] * Wo[cl + CG:] * so[None, :]).astype(NPBF16),
        b1=np.concatenate([b1p, b1p], axis=0),
        b2=np.concatenate([b2p, b2p], axis=0),
        bo=np.concatenate([to, to]).astype(np.float32).reshape(128, 1),
    )


def _col_maps():
    """E column c -> (target, k).  Device consumption order per 2048-col
    step i: matmul m = 2*b2 + h covers cols i*2048 + m*512 + u*16 + k and
    lands at psum (partition h*64+ch, bank b2, u); pooled target index is
    t = h*HALF_T + i*64 + b2*32 + u."""
    c = np.arange(COLS)
    g = c // STEP_C
    m = (c // 512) % 4
    u = (c // K) % 32
    k = c % K
    t = (m % 2) * HALF_T + g * 64 + (m // 2) * 32 + u
    return t.astype(np.int64), k.astype(np.int64)


_T_OF_C, _K_OF_C = _col_maps()


def _host_prep(feat_s1, feat_s2, feat_last, Wg1, bn_g1, Wg2, bn_g2,
               Wout, bn_out, idx_s1, idx_s2):
    common = host_prep_weights(Wg1, bn_g1, Wg2, bn_g2, Wout, bn_out)

    f1b = feat_s1.astype(NPBF16)
    f2b = feat_s2.astype(NPBF16)
    n = feat_last.shape[0]
    n_shard = n // N_CORES

    in_maps = []
    for core in range(N_CORES):
        lo, hi = core * n_shard, (core + 1) * n_shard
        i1 = np.zeros((NT, K), np.int64)
        i1[:n_shard] = idx_s1[lo:hi]
        i2 = np.zeros((NT, K), np.int64)
        i2[:n_shard] = idx_s2[lo:hi]
        E = np.empty((ECH, COLS), NPBF16)
        E[:C2] = f2b[i2[_T_OF_C, _K_OF_C]].T
        E[C2:] = f1b[i1[_T_OF_C, _K_OF_C]].T
        flT = np.zeros((NT, CL), np.float32)
        flT[:n_shard] = feat_last[lo:hi]
        in_maps.append(dict(
            common, E=E, fLT=np.ascontiguousarray(flT.T.astype(NPBF16))))
    return in_maps, n_shard


_BUILD_CACHE = {}


def _ensure_profile_hook():
    """This image's ``antenv`` lacks ``axon_hooks``; concourse's trace=True
    path imports it unconditionally. Provide the module and install the
    ctypes NTFF hook against libaxon_pjrt.so (mirrors trn_boot.py)."""
    import types
    import ctypes
    import contextlib

    try:
        from antenv.axon_hooks import get_axon_ntff_profile_hook  # noqa: F401
        return
    except ImportError:
        pass

    mod = types.ModuleType("antenv.axon_hooks")
    mod._hook = None
    mod.set_axon_ntff_profile_hook = lambda h: setattr(mod, "_hook", h)
    mod.get_axon_ntff_profile_hook = lambda: mod._hook
    sys.modules["antenv.axon_hooks"] = mod
    import antenv
    antenv.axon_hooks = mod

    so_path = "/opt/axon/libaxon_pjrt.so"
    if not os.path.exists(so_path):
        return
    lib = ctypes.CDLL(so_path)
    if not hasattr(lib, "axon_start_nrt_profile"):
        return
    lib.axon_start_nrt_profile.argtypes = [
        ctypes.POINTER(ctypes.c_int64), ctypes.c_size_t,
    ]
    lib.axon_start_nrt_profile.restype = ctypes.c_int64
    lib.axon_stop_nrt_profile.argtypes = [ctypes.c_char_p]
    lib.axon_stop_nrt_profile.restype = ctypes.c_int64

    @contextlib.contextmanager
    def _hook(output_dir, device_ids):
        import jax
        jax.devices()
        if device_ids:
            ids = (ctypes.c_int64 * len(device_ids))(*device_ids)
            rc = lib.axon_start_nrt_profile(ids, len(device_ids))
        else:
            rc = lib.axon_start_nrt_profile(None, 0)
        if rc != 0:
            raise RuntimeError(f"axon_start_nrt_profile rc={rc}")
        try:
            yield
        finally:
            nf = lib.axon_stop_nrt_profile(str(output_dir).encode())
            print(f"profile: {nf} file(s) written to {output_dir}",
                  file=sys.stderr)

    mod.set_axon_ntff_profile_hook(_hook)


def kernel(**inputs):
    from concourse import bass_utils
    from concourse.bass_interp import get_hw_module

    in_maps, n_shard = _host_prep(**inputs)
    if "nc" not in _BUILD_CACHE:
        _BUILD_CACHE["nc"] = build()
    nc = _BUILD_CACHE["nc"]

    old_m = nc.m
    nc.m = get_hw_module(nc.m)
    try:
        trace = os.environ.get("BIGNN_TRACE", "0") == "1"
        if trace:
            _ensure_profile_hook()
        res = bass_utils.run_bass_kernel_spmd(
            nc, in_maps, core_ids=list(range(N_CORES)),
            trace=trace,
            trace_cores=list(range(N_CORES)) if trace else None,
        )
    finally:
        nc.m = old_m

    kernel.last_results = res
    n = inputs["feat_last"].shape[0]
    out = np.empty((n, CG), np.float32)
    for c in range(N_CORES):
        oT = res.results[c]["oT"].astype(np.float32)
        full = np.concatenate([oT[:CG, :].T, oT[CG:, :].T], axis=0)
        out[c * n_shard:(c + 1) * n_shard] = full[:n_shard]
    return out
